# revision 29
# baseline (speedup 1.0000x reference)
"""BatchRGATLayer Trainium2 kernel (8 NeuronCores, data-parallel over (batch, row-half)).

kernel(**inputs) takes FULL inputs (x, edge, adj, W, W1, a), shards across 8
cores (core c -> batch c//2, rows (c%2)*256 .. +256), runs one SPMD Bass
program on all 8 cores, gathers to the full (4, 512, 256) output.

For row-half cores (c%2==1) the node axis is rolled by -256 on the host for
x, edge(j), adj(j) so the single SPMD program can treat local rows as [0,256).
Softmax and att@h are invariant to a consistent j-permutation.

Device algorithm per core (rows R=256 of one batch):
  h = x @ W (PE);  s_i = hT @ a1 (PE), s_j = a2T @ hT (PE)
  s_e[i,j] = sum_e edge[i,j,e] * (W1@a3)[e]  -- dominant stream, on DVE:
    edge is DMA'd with inline fp32->fp16 cast (SWDGE), multiplied by the
    broadcast w-vector in 2x mode, then segment-summed by a 3-level fp16
    pairwise-add tree + an 8-wide reduce (tensor_reduce measures 1x-only
    on HW regardless of dtype, so the tree stays).
  softmax over j without max-subtraction (logits are small; exp gets a
  -11 bias to stay in fp16 range; adj<=0 handled by multiplicative mask).
  h' = att @ h via PE (fp16), out = elu(h'/denom) = max(exp(min(x,0))-1, x).

ALL DMA (inputs, edge stream, output stores) rides the single SWDGE (gpsimd)
queue: concurrent HWDGE-queue traffic degrades DMA engine 79 per-packet, and
starved hardware-queue inputs stall every DVE consumer. Queue order: a + W1
(wcol deps, needed within ~10us), first two edge tiles, then x/W/adj, then
the remaining edge flood; out stores are emitted after the loop. acol6 is
derived on-chip from a_all via PE transposes instead of a 768-descriptor DMA.
Uniform 128-wide j-blocks only: mixed tile widths make the Tile scheduler
mis-order the DVE queue. Never alias DVE operands in-place (intermittent
corruption measured on HW).
"""

import sys

sys.path.insert(0, "/opt/trn_rl_repo")

from contextlib import ExitStack

import numpy as np

import concourse.bass as bass
import concourse.tile as tile
from concourse import bacc, mybir
from concourse.bass_utils import run_bass_kernel_spmd
from concourse.masks import make_identity

F32 = mybir.dt.float32
F16 = mybir.dt.float16
AF = mybir.ActivationFunctionType
ALU = mybir.AluOpType

# problem dims (hardcoded per spec)
B, N, IN_F, E_F, OUT_F = 4, 512, 256, 64, 256
R = 256
N_CORES = 8
ALPHA = 0.2
EXP_BIAS = -11.0

# j-blocks (offset, width) per row-block; it=1 ends with two half tiles
BLOCKS = [
    [(0, 128), (128, 128), (256, 128), (384, 128)],
    [(0, 128), (128, 128), (256, 128), (384, 128)],
]
ED_BUFS = 6

_CACHE = {}


def build_program():
    nc = bacc.Bacc("TRN2", target_bir_lowering=False, debug=False)

    edge_d = nc.dram_tensor("edge_s", [R, N, E_F], F32, kind="ExternalInput").ap()
    adj_d = nc.dram_tensor("adj_s", [R, N], F32, kind="ExternalInput").ap()
    x_d = nc.dram_tensor("x_b", [N, IN_F], F32, kind="ExternalInput").ap()
    w_d = nc.dram_tensor("W", [IN_F, OUT_F], F32, kind="ExternalInput").ap()
    w1_d = nc.dram_tensor("W1", [E_F, OUT_F], F32, kind="ExternalInput").ap()
    a_d = nc.dram_tensor("a", [3 * OUT_F, 1], F32, kind="ExternalInput").ap()
    out_d = nc.dram_tensor("out_s", [R, OUT_F], F32, kind="ExternalOutput").ap()

    NIT = R // 128
    NJT = N // 128
    NFT = IN_F // 128
    NOT_ = OUT_F // 128

    ctx = ExitStack()
    with tile.TileContext(nc) as tc, ctx:
        consts = ctx.enter_context(tc.tile_pool(name="consts", bufs=1))
        sb1 = ctx.enter_context(tc.tile_pool(name="sb1", bufs=1))
        psx = ctx.enter_context(tc.tile_pool(name="psx", bufs=2, space="PSUM"))
        ed_pool = ctx.enter_context(tc.tile_pool(name="ed", bufs=ED_BUFS))
        prod_pool = ctx.enter_context(tc.tile_pool(name="prod", bufs=1))
        soft_pool = ctx.enter_context(tc.tile_pool(name="soft", bufs=2))
        att_ps_pool = ctx.enter_context(tc.tile_pool(name="att_ps", bufs=2, space="PSUM"))
        attT_pool = ctx.enter_context(tc.tile_pool(name="attT", bufs=2))
        hp_ps_pool = ctx.enter_context(tc.tile_pool(name="hp_ps", bufs=2, space="PSUM"))
        out_pool = ctx.enter_context(tc.tile_pool(name="outp", bufs=2))

        # ---- persistent tiles ----
        ident = consts.tile([128, 128], F32)
        ident16 = consts.tile([128, 128], F16)
        ones_row = consts.tile([1, 128], F32)
        expbias = consts.tile([128, 1], F32)
        w_all = consts.tile([128, NFT * OUT_F], F32)
        w_sb = [w_all[:, bass.ts(ft, OUT_F)] for ft in range(NFT)]
        w1_sb = consts.tile([E_F, OUT_F], F32)
        a_all = consts.tile([1, 3 * OUT_F], F32)
        a3_row = a_all[:, 2 * OUT_F : 3 * OUT_F]
        acol6 = consts.tile([128, 6], F32)
        a1_col = [acol6[:, 0 + ot : 1 + ot] for ot in range(NOT_)]
        a2_col = [acol6[:, 2 + ot : 3 + ot] for ot in range(NOT_)]
        x_all = sb1.tile([128, NJT * IN_F], F32)
        x_sb = [x_all[:, bass.ts(rt, IN_F)] for rt in range(NJT)]
        adj_all = sb1.tile([128, NIT * N], F32)
        adj_sb = [adj_all[:, bass.ts(it, N)] for it in range(NIT)]
        xT_sb = [sb1.tile([128, N], F32, tag=f"xT{ft}", name=f"xT{ft}") for ft in range(NFT)]
        h_sb = [sb1.tile([128, OUT_F], F32, tag=f"h{rt}", name=f"h{rt}") for rt in range(NJT)]
        h16_sb = [sb1.tile([128, OUT_F], F16, tag=f"h16_{rt}", name=f"h16_{rt}") for rt in range(NJT)]
        hT_sb = [sb1.tile([128, N], F32, tag=f"hT{ot}", name=f"hT{ot}") for ot in range(NOT_)]
        mask_sb = [sb1.tile([128, N], F16, tag=f"mk{it}", name=f"mk{it}") for it in range(NIT)]
        si_col = [sb1.tile([128, 1], F32, tag=f"si{it}", name=f"si{it}") for it in range(NIT)]
        se_dve = [sb1.tile([128, N], F16, tag=f"se{it}", name=f"se{it}") for it in range(NIT)]
        sj_rep = sb1.tile([128, N], F16)
        b_sb = [sb1.tile([128, N], F16, tag=f"b{it}", name=f"b{it}") for it in range(NIT)]
        wcol = consts.tile([128, E_F], F16)
        w1a3 = consts.tile([E_F, 1], F32)
        w1a3_row = consts.tile([1, E_F], F32)

        # ---- ALL DMA rides the SWDGE (gpsimd) queue: any concurrent traffic
        # on the HWDGE queues degrades DMA engine 79 (their queue manager)
        # by ~24% per packet (measured), stretching the edge stream span.
        # Order: wcol deps (a, W1) first, then the first two edge tiles,
        # then the remaining inputs, then the rest of the edge flood. ----
        nc.gpsimd.dma_start(a_all[:], a_d[:, :].rearrange("a b -> b a"))
        nc.gpsimd.dma_start(w1_sb[:], w1_d[:, :])

        def input_dmas():
            nc.gpsimd.dma_start(
                x_all[:].rearrange("p (rt f) -> p rt f", f=IN_F),
                x_d[:, :].rearrange("(rt p) f -> p rt f", p=128),
            )
            nc.gpsimd.dma_start(
                w_all[:].rearrange("p (ft f) -> p ft f", f=OUT_F),
                w_d[:, :].rearrange("(ft p) f -> p ft f", p=128),
            )
            nc.gpsimd.dma_start(
                adj_all[:].rearrange("p (it j) -> p it j", j=N),
                adj_d[:, :].rearrange("(it p) j -> p it j", p=128),
            )

        # ---- setup part 1: constants the edge stream needs (wcol) ----
        nc.gpsimd.memset(ones_row[:], 1.0)
        nc.gpsimd.memset(expbias[:], EXP_BIAS)
        make_identity(nc, ident)
        make_identity(nc, ident16)

        # acol6 on-chip from a_all: column c of acol6 is a[c*128:(c+1)*128]
        acol_ps = psx.tile([128, 8], F32, tag="acps")
        for c in range(6):
            nc.tensor.transpose(
                acol_ps[:, c : c + 1], a_all[:, bass.ts(c, 128)], ident[0:1, 0:1]
            )
        nc.scalar.copy(acol6[:], acol_ps[:, 0:6])

        a3_rep_ps = psx.tile([E_F, OUT_F], F32, tag="mps")
        nc.tensor.matmul(a3_rep_ps[:], ones_row[:, 0:E_F], a3_row[:])
        a3_rep = sb1.tile([E_F, OUT_F], F32)
        nc.scalar.copy(a3_rep[:], a3_rep_ps[:])
        ttr_scratch = sb1.tile([E_F, OUT_F], F32)
        nc.vector.tensor_tensor(ttr_scratch[:], w1_sb[:], a3_rep[:], ALU.mult)
        nc.vector.reduce_sum(w1a3[:], ttr_scratch[:], axis=mybir.AxisListType.X)
        w1a3_row_ps = psx.tile([1, E_F], F32, tag="mps")
        nc.tensor.transpose(w1a3_row_ps[:], w1a3[:], ident[0:E_F, 0:E_F])
        nc.scalar.copy(w1a3_row[:], w1a3_row_ps[:])
        wcol_ps = psx.tile([128, E_F], F32, tag="mps")
        nc.tensor.matmul(wcol_ps[:], ones_row[:], w1a3_row[:])
        nc.scalar.copy(wcol[:], wcol_ps[:])

        def setup2():
            # xT via PE transposes
            for rt in range(NJT):
                xt_ps = psx.tile([128, NFT * 128], F32, tag="mps")
                for ft in range(NFT):
                    nc.tensor.transpose(
                        xt_ps[:, bass.ts(ft, 128)], x_sb[rt][:, bass.ts(ft, 128)], ident[:]
                    )
                for ft in range(NFT):
                    nc.scalar.copy(xT_sb[ft][:, bass.ts(rt, 128)], xt_ps[:, bass.ts(ft, 128)])
            # h = x @ W
            for rt in range(NJT):
                h_ps = psx.tile([128, OUT_F], F32, tag="mps")
                for ft in range(NFT):
                    nc.tensor.matmul(
                        h_ps[:], xT_sb[ft][:, bass.ts(rt, 128)], w_sb[ft][:],
                        start=(ft == 0), stop=(ft == NFT - 1),
                    )
                nc.scalar.copy(h_sb[rt][:], h_ps[:])
            for rt in range(NJT):
                nc.scalar.copy(h16_sb[rt][:], h_sb[rt][:])
            # hT = W^T x^T
            for ot in range(NOT_):
                for rt in range(NJT):
                    ht_ps = psx.tile([128, 128], F32, tag="mps")
                    for ft in range(NFT):
                        nc.tensor.matmul(
                            ht_ps[:],
                            w_sb[ft][:, bass.ts(ot, 128)],
                            xT_sb[ft][:, bass.ts(rt, 128)],
                            start=(ft == 0), stop=(ft == NFT - 1),
                        )
                    nc.scalar.copy(hT_sb[ot][:, bass.ts(rt, 128)], ht_ps[:])
            # s_i for local rows via PE: si[i] = sum_o hT[o,i] * a1[o]
            for it in range(NIT):
                si_ps = psx.tile([128, 1], F32, tag="mps")
                for ot in range(NOT_):
                    nc.tensor.matmul(
                        si_ps[:], hT_sb[ot][:, bass.ts(it, 128)], a1_col[ot][:],
                        start=(ot == 0), stop=(ot == NOT_ - 1),
                    )
                nc.scalar.copy(si_col[it][:], si_ps[:])
            # s_j for all nodes, replicated across partitions
            sj_ps = psx.tile([1, N], F32, tag="mps")
            for ot in range(NOT_):
                nc.tensor.matmul(
                    sj_ps[:], a2_col[ot][:], hT_sb[ot][:],
                    start=(ot == 0), stop=(ot == NOT_ - 1),
                )
            sj_row = sb1.tile([1, N], F32)
            nc.scalar.copy(sj_row[:], sj_ps[:])
            sjrep_ps = psx.tile([128, N], F32, tag="mps")
            nc.tensor.matmul(sjrep_ps[:], ones_row[:], sj_row[:])
            nc.scalar.copy(sj_rep[:], sjrep_ps[:])
            # masks (1.0 where adj > 0) and B = sj + si per row-block;
            # on Pool: these mid-stream DVE inserts were eating the
            # 0.45us/tile DVE-vs-DMA slack and backlogging the tail
            for it in range(NIT):
                nc.gpsimd.tensor_scalar(
                    mask_sb[it][:], adj_sb[it][:], 0.0, None, op0=ALU.is_gt
                )
                nc.gpsimd.tensor_scalar(
                    b_sb[it][:], sj_rep[:], si_col[it][:], None, op0=ALU.add
                )

        n_ed_issued = 0
        ot_tiles = []
        for it in range(NIT):
            # ---- edge stream: s_e via fp16 mult + pairwise tree ----
            for (j0, jw) in BLOCKS[it]:
                if n_ed_issued == 2:
                    input_dmas()
                n_ed_issued += 1
                ed = ed_pool.tile(
                    [128, jw * E_F], F16, tag=f"ed{jw}", name=f"ed{jw}",
                    bufs=(ED_BUFS if jw == 128 else 2),
                )
                # the very last tile is the serial tail: land it as two
                # half DMAs into the same buffer so the first half's DVE
                # chain starts ~5us before the full tile has landed
                # (subtile deps track the halves independently)
                last_tile = (it == NIT - 1) and (j0 + jw == N)
                halves = 2 if last_tile else 1
                hw_ = jw // halves
                for hb in range(halves):
                    nc.gpsimd.dma_start(
                        ed[:, hb * hw_ * E_F : (hb + 1) * hw_ * E_F],
                        edge_d[
                            bass.ts(it, 128), j0 + hb * hw_ : j0 + (hb + 1) * hw_, :
                        ].rearrange("p a b -> p (a b)"),
                    )
                prod = prod_pool.tile(
                    [128, jw * E_F], F16, tag=f"pr{jw}", name=f"pr{jw}"
                )
                t1 = prod_pool.tile([128, jw * 32], F16, tag=f"t1_{jw}", name=f"t1_{jw}")
                t2 = prod_pool.tile([128, jw * 16], F16, tag=f"t2_{jw}", name=f"t2_{jw}")
                t3 = prod_pool.tile([128, jw * 8], F16, tag=f"t3_{jw}", name=f"t3_{jw}")
                for hb in range(halves):
                    sl = slice(hb * hw_, (hb + 1) * hw_)
                    v0 = prod[:].rearrange("p (a b) -> p a b", b=E_F)[:, sl]
                    e0 = ed[:].rearrange("p (a b) -> p a b", b=E_F)[:, sl]
                    nc.vector.tensor_tensor(
                        v0, e0,
                        wcol[:, None, :].broadcast_to([128, hw_, E_F]),
                        ALU.mult,
                    )
                    v1 = t1[:].rearrange("p (a b) -> p a b", b=32)[:, sl]
                    nc.vector.tensor_tensor(v1, v0[:, :, 0:32], v0[:, :, 32:64], ALU.add)
                    v2 = t2[:].rearrange("p (a b) -> p a b", b=16)[:, sl]
                    nc.vector.tensor_tensor(v2, v1[:, :, 0:16], v1[:, :, 16:32], ALU.add)
                    v3 = t3[:].rearrange("p (a b) -> p a b", b=8)[:, sl]
                    nc.vector.tensor_tensor(v3, v2[:, :, 0:8], v2[:, :, 8:16], ALU.add)
                    with nc.allow_low_precision(reason="fp16 sum of 8 small partials"):
                        nc.vector.reduce_sum(
                            se_dve[it][:, j0 + hb * hw_ : j0 + (hb + 1) * hw_],
                            v3,
                            axis=mybir.AxisListType.X,
                        )

            if it == 0:
                setup2()

            # ---- softmax (no max-subtraction; exp biased into fp16 range) ----
            z = soft_pool.tile([128, N], F16, tag="z", bufs=1)
            nc.vector.tensor_tensor(z[:], se_dve[it][:], b_sb[it][:], ALU.add)
            zl = soft_pool.tile([128, N], F16, tag="zl", bufs=1)
            nc.vector.scalar_tensor_tensor(
                out=zl[:], in0=z[:], scalar=ALPHA, in1=z[:], op0=ALU.mult, op1=ALU.max
            )
            p = soft_pool.tile([128, N], F16, tag="p")
            nc.scalar.activation(p[:], zl[:], AF.Exp, bias=expbias[:])
            pm = soft_pool.tile([128, N], F16, tag="pm")
            # it0's pm rides Pool (mid-stream DVE relief); it1's stays on
            # DVE (tail-serial, Pool hop would lengthen the critical path)
            nc.vector.tensor_tensor(pm[:], p[:], mask_sb[it][:], ALU.mult)
            denom = soft_pool.tile([128, 1], F32, tag="den")
            nc.vector.reduce_sum(denom[:], pm[:], axis=mybir.AxisListType.X)
            rden = soft_pool.tile([128, 1], F32, tag="rden")
            nc.vector.reciprocal(rden[:], denom[:])

            # ---- h' = att @ h (fp16 PE path) ----
            hp_ps = hp_ps_pool.tile([128, OUT_F], F32)
            for jt in range(NJT):
                aps = att_ps_pool.tile([128, 128], F16)
                nc.tensor.transpose(aps[:], pm[:, bass.ts(jt, 128)], ident16[:])
                asb = attT_pool.tile([128, 128], F16)
                nc.scalar.copy(asb[:], aps[:])
                nc.tensor.matmul(
                    hp_ps[:], asb[:], h16_sb[jt][:],
                    start=(jt == 0), stop=(jt == NJT - 1),
                )

            # ---- normalize + ELU + store ----
            xx = out_pool.tile([128, OUT_F], F32, tag="xx", bufs=1)
            nc.scalar.mul(xx[:], hp_ps[:], rden[:])
            tmin = out_pool.tile([128, OUT_F], F32, tag="tm", bufs=1)
            nc.vector.tensor_scalar(tmin[:], xx[:], 0.0, None, op0=ALU.min)
            ex = out_pool.tile([128, OUT_F], F32, tag="ex", bufs=1)
            nc.scalar.activation(ex[:], tmin[:], AF.Exp)
            ot_sb = out_pool.tile([128, OUT_F], F32, tag="ot")
            nc.vector.scalar_tensor_tensor(
                out=ot_sb[:], in0=ex[:], scalar=-1.0, in1=xx[:], op0=ALU.add, op1=ALU.max
            )
            ot_tiles.append((it, ot_sb))

        # out stores ride the SWDGE queue too (a HWDGE store mid-span would
        # re-trigger the engine-79 queue-manager slowdown)
        for it, ot_sb in ot_tiles:
            nc.gpsimd.dma_start(out_d[bass.ts(it, 128), :], ot_sb[:])

    nc.compile()
    return nc


def _shard(x, edge, adj, W, W1, a):
    in_maps = []
    for c in range(N_CORES):
        bi, half = c // 2, c % 2
        r0 = half * R
        if r0:
            xb = np.roll(x[bi], -r0, axis=0)
            ed = np.roll(edge[bi, r0 : r0 + R], -r0, axis=1)
            ad = np.roll(adj[bi, r0 : r0 + R], -r0, axis=1)
        else:
            xb = x[bi]
            ed = edge[bi, 0:R]
            ad = adj[bi, 0:R]
        in_maps.append(
            {
                "edge_s": np.ascontiguousarray(ed),
                "adj_s": np.ascontiguousarray(ad),
                "x_b": np.ascontiguousarray(xb),
                "W": W,
                "W1": W1,
                "a": a,
            }
        )
    return in_maps


def kernel(x, edge, adj, W, W1, a, _trace=False):
    if "nc" not in _CACHE:
        _CACHE["nc"] = build_program()
    nc = _CACHE["nc"]

    x = np.asarray(x, dtype=np.float32)
    edge = np.asarray(edge, dtype=np.float32)
    adj = np.asarray(adj, dtype=np.float32)
    W = np.ascontiguousarray(np.asarray(W, dtype=np.float32))
    W1 = np.ascontiguousarray(np.asarray(W1, dtype=np.float32))
    a = np.ascontiguousarray(np.asarray(a, dtype=np.float32).reshape(3 * OUT_F, 1))

    in_maps = _shard(x, edge, adj, W, W1, a)
    res = run_bass_kernel_spmd(nc, in_maps, core_ids=list(range(N_CORES)), trace=_trace)
    out = np.empty((B, N, OUT_F), dtype=np.float32)
    for c in range(N_CORES):
        bi, half = c // 2, c % 2
        out[bi, half * R : (half + 1) * R] = res.results[c]["out_s"]
    if _trace:
        _CACHE["last_exec_time_ns"] = res.exec_time_ns
        _CACHE["last_res"] = res
    return out


# revision 30
# speedup vs baseline: 1.0200x; 1.0200x over previous
"""BatchRGATLayer Trainium2 kernel (8 NeuronCores, data-parallel over (batch, row-half)).

kernel(**inputs) takes FULL inputs (x, edge, adj, W, W1, a), shards across 8
cores (core c -> batch c//2, rows (c%2)*256 .. +256), runs one SPMD Bass
program on all 8 cores, gathers to the full (4, 512, 256) output.

For row-half cores (c%2==1) the node axis is rolled by -256 on the host for
x, edge(j), adj(j) so the single SPMD program can treat local rows as [0,256).
Softmax and att@h are invariant to a consistent j-permutation.

Device algorithm per core (rows R=256 of one batch):
  h = x @ W (PE);  s_i = hT @ a1 (PE), s_j = a2T @ hT (PE)
  s_e[i,j] = sum_e edge[i,j,e] * (W1@a3)[e]  -- dominant stream, on DVE:
    edge is DMA'd with inline fp32->fp16 cast (SWDGE), multiplied by the
    broadcast w-vector in 2x mode, then segment-summed by a 3-level fp16
    pairwise-add tree + an 8-wide reduce (tensor_reduce measures 1x-only
    on HW regardless of dtype, so the tree stays).
  softmax over j without max-subtraction (logits are small; exp gets a
  -11 bias to stay in fp16 range; adj<=0 handled by multiplicative mask).
  h' = att @ h via PE (fp16), out = elu(h'/denom) = max(exp(min(x,0))-1, x).

ALL DMA (inputs, edge stream, output stores) rides the single SWDGE (gpsimd)
queue: concurrent HWDGE-queue traffic degrades DMA engine 79 per-packet, and
starved hardware-queue inputs stall every DVE consumer. Queue order: a + W1
(wcol deps, needed within ~10us), first two edge tiles, then x/W/adj, then
the remaining edge flood; out stores are emitted after the loop. acol6 is
derived on-chip from a_all via PE transposes instead of a 768-descriptor DMA.
Uniform 128-wide j-blocks only: mixed tile widths make the Tile scheduler
mis-order the DVE queue. Never alias DVE operands in-place (intermittent
corruption measured on HW).
"""

import sys

sys.path.insert(0, "/opt/trn_rl_repo")

from contextlib import ExitStack

import numpy as np

import concourse.bass as bass
import concourse.tile as tile
from concourse import bacc, mybir
from concourse.bass_utils import run_bass_kernel_spmd
from concourse.masks import make_identity

F32 = mybir.dt.float32
F16 = mybir.dt.float16
AF = mybir.ActivationFunctionType
ALU = mybir.AluOpType

# problem dims (hardcoded per spec)
B, N, IN_F, E_F, OUT_F = 4, 512, 256, 64, 256
R = 256
N_CORES = 8
ALPHA = 0.2
EXP_BIAS = -11.0

# j-blocks (offset, width) per row-block; it=1 ends with two half tiles
BLOCKS = [
    [(0, 128), (128, 128), (256, 128), (384, 128)],
    [(0, 128), (128, 128), (256, 128), (384, 128)],
]
ED_BUFS = 6

_CACHE = {}


def build_program():
    nc = bacc.Bacc("TRN2", target_bir_lowering=False, debug=False)

    edge_d = nc.dram_tensor("edge_s", [R, N, E_F], F32, kind="ExternalInput").ap()
    adj_d = nc.dram_tensor("adj_s", [R, N], F32, kind="ExternalInput").ap()
    x_d = nc.dram_tensor("x_b", [N, IN_F], F32, kind="ExternalInput").ap()
    w_d = nc.dram_tensor("W", [IN_F, OUT_F], F32, kind="ExternalInput").ap()
    w1_d = nc.dram_tensor("W1", [E_F, OUT_F], F32, kind="ExternalInput").ap()
    a_d = nc.dram_tensor("a", [3 * OUT_F, 1], F32, kind="ExternalInput").ap()
    out_d = nc.dram_tensor("out_s", [R, OUT_F], F32, kind="ExternalOutput").ap()

    NIT = R // 128
    NJT = N // 128
    NFT = IN_F // 128
    NOT_ = OUT_F // 128

    ctx = ExitStack()
    with tile.TileContext(nc) as tc, ctx:
        consts = ctx.enter_context(tc.tile_pool(name="consts", bufs=1))
        sb1 = ctx.enter_context(tc.tile_pool(name="sb1", bufs=1))
        psx = ctx.enter_context(tc.tile_pool(name="psx", bufs=2, space="PSUM"))
        ed_pool = ctx.enter_context(tc.tile_pool(name="ed", bufs=ED_BUFS))
        prod_pool = ctx.enter_context(tc.tile_pool(name="prod", bufs=1))
        soft_pool = ctx.enter_context(tc.tile_pool(name="soft", bufs=2))
        att_ps_pool = ctx.enter_context(tc.tile_pool(name="att_ps", bufs=2, space="PSUM"))
        attT_pool = ctx.enter_context(tc.tile_pool(name="attT", bufs=2))
        hp_ps_pool = ctx.enter_context(tc.tile_pool(name="hp_ps", bufs=2, space="PSUM"))
        out_pool = ctx.enter_context(tc.tile_pool(name="outp", bufs=2))

        # ---- persistent tiles ----
        ident = consts.tile([128, 128], F32)
        ident16 = consts.tile([128, 128], F16)
        ones_row = consts.tile([1, 128], F32)
        expbias = consts.tile([128, 1], F32)
        w_all = consts.tile([128, NFT * OUT_F], F32)
        w_sb = [w_all[:, bass.ts(ft, OUT_F)] for ft in range(NFT)]
        w1_sb = consts.tile([E_F, OUT_F], F32)
        a_all = consts.tile([1, 3 * OUT_F], F32)
        a3_row = a_all[:, 2 * OUT_F : 3 * OUT_F]
        acol6 = consts.tile([128, 6], F32)
        a1_col = [acol6[:, 0 + ot : 1 + ot] for ot in range(NOT_)]
        a2_col = [acol6[:, 2 + ot : 3 + ot] for ot in range(NOT_)]
        x_all = sb1.tile([128, NJT * IN_F], F32)
        x_sb = [x_all[:, bass.ts(rt, IN_F)] for rt in range(NJT)]
        adj_all = sb1.tile([128, NIT * N], F32)
        adj_sb = [adj_all[:, bass.ts(it, N)] for it in range(NIT)]
        xT_sb = [sb1.tile([128, N], F32, tag=f"xT{ft}", name=f"xT{ft}") for ft in range(NFT)]
        h_sb = [sb1.tile([128, OUT_F], F32, tag=f"h{rt}", name=f"h{rt}") for rt in range(NJT)]
        h16_sb = [sb1.tile([128, OUT_F], F16, tag=f"h16_{rt}", name=f"h16_{rt}") for rt in range(NJT)]
        hT_sb = [sb1.tile([128, N], F32, tag=f"hT{ot}", name=f"hT{ot}") for ot in range(NOT_)]
        mask_sb = [sb1.tile([128, N], F16, tag=f"mk{it}", name=f"mk{it}") for it in range(NIT)]
        si_col = [sb1.tile([128, 1], F32, tag=f"si{it}", name=f"si{it}") for it in range(NIT)]
        se_dve = [sb1.tile([128, N], F16, tag=f"se{it}", name=f"se{it}") for it in range(NIT)]
        sj_rep = sb1.tile([128, N], F16)
        b_sb = [sb1.tile([128, N], F16, tag=f"b{it}", name=f"b{it}") for it in range(NIT)]
        wcol = consts.tile([128, E_F], F16)
        w1a3 = consts.tile([E_F, 1], F32)
        w1a3_row = consts.tile([1, E_F], F32)

        # ---- ALL DMA rides the SWDGE (gpsimd) queue: any concurrent traffic
        # on the HWDGE queues degrades DMA engine 79 (their queue manager)
        # by ~24% per packet (measured), stretching the edge stream span.
        # Order: wcol deps (a, W1) first, then the first two edge tiles,
        # then the remaining inputs, then the rest of the edge flood. ----
        pre_ed = ed_pool.tile([128, 128 * E_F], F16, tag="ed128", name="ed128", bufs=ED_BUFS)
        nc.gpsimd.dma_start(
            pre_ed[:],
            edge_d[0:128, 0:128, :].rearrange("p a b -> p (a b)"),
        )
        nc.gpsimd.dma_start(a_all[:], a_d[:, :].rearrange("a b -> b a"))
        nc.gpsimd.dma_start(w1_sb[:], w1_d[:, :])

        def input_dmas():
            nc.gpsimd.dma_start(
                x_all[:].rearrange("p (rt f) -> p rt f", f=IN_F),
                x_d[:, :].rearrange("(rt p) f -> p rt f", p=128),
            )
            nc.gpsimd.dma_start(
                w_all[:].rearrange("p (ft f) -> p ft f", f=OUT_F),
                w_d[:, :].rearrange("(ft p) f -> p ft f", p=128),
            )
            nc.gpsimd.dma_start(
                adj_all[:].rearrange("p (it j) -> p it j", j=N),
                adj_d[:, :].rearrange("(it p) j -> p it j", p=128),
            )

        # ---- setup part 1: constants the edge stream needs (wcol) ----
        nc.gpsimd.memset(ones_row[:], 1.0)
        nc.gpsimd.memset(expbias[:], EXP_BIAS)
        make_identity(nc, ident)
        make_identity(nc, ident16)

        # acol6 on-chip from a_all: column c of acol6 is a[c*128:(c+1)*128]
        acol_ps = psx.tile([128, 8], F32, tag="acps")
        for c in range(6):
            nc.tensor.transpose(
                acol_ps[:, c : c + 1], a_all[:, bass.ts(c, 128)], ident[0:1, 0:1]
            )
        nc.scalar.copy(acol6[:], acol_ps[:, 0:6])

        a3_rep_ps = psx.tile([E_F, OUT_F], F32, tag="mps")
        nc.tensor.matmul(a3_rep_ps[:], ones_row[:, 0:E_F], a3_row[:])
        a3_rep = sb1.tile([E_F, OUT_F], F32)
        nc.scalar.copy(a3_rep[:], a3_rep_ps[:])
        ttr_scratch = sb1.tile([E_F, OUT_F], F32)
        nc.vector.tensor_tensor(ttr_scratch[:], w1_sb[:], a3_rep[:], ALU.mult)
        nc.vector.reduce_sum(w1a3[:], ttr_scratch[:], axis=mybir.AxisListType.X)
        w1a3_row_ps = psx.tile([1, E_F], F32, tag="mps")
        nc.tensor.transpose(w1a3_row_ps[:], w1a3[:], ident[0:E_F, 0:E_F])
        nc.scalar.copy(w1a3_row[:], w1a3_row_ps[:])
        wcol_ps = psx.tile([128, E_F], F32, tag="mps")
        nc.tensor.matmul(wcol_ps[:], ones_row[:], w1a3_row[:])
        nc.scalar.copy(wcol[:], wcol_ps[:])

        def setup2():
            # xT via PE transposes
            for rt in range(NJT):
                xt_ps = psx.tile([128, NFT * 128], F32, tag="mps")
                for ft in range(NFT):
                    nc.tensor.transpose(
                        xt_ps[:, bass.ts(ft, 128)], x_sb[rt][:, bass.ts(ft, 128)], ident[:]
                    )
                for ft in range(NFT):
                    nc.scalar.copy(xT_sb[ft][:, bass.ts(rt, 128)], xt_ps[:, bass.ts(ft, 128)])
            # h = x @ W
            for rt in range(NJT):
                h_ps = psx.tile([128, OUT_F], F32, tag="mps")
                for ft in range(NFT):
                    nc.tensor.matmul(
                        h_ps[:], xT_sb[ft][:, bass.ts(rt, 128)], w_sb[ft][:],
                        start=(ft == 0), stop=(ft == NFT - 1),
                    )
                nc.scalar.copy(h_sb[rt][:], h_ps[:])
            for rt in range(NJT):
                nc.scalar.copy(h16_sb[rt][:], h_sb[rt][:])
            # hT = W^T x^T
            for ot in range(NOT_):
                for rt in range(NJT):
                    ht_ps = psx.tile([128, 128], F32, tag="mps")
                    for ft in range(NFT):
                        nc.tensor.matmul(
                            ht_ps[:],
                            w_sb[ft][:, bass.ts(ot, 128)],
                            xT_sb[ft][:, bass.ts(rt, 128)],
                            start=(ft == 0), stop=(ft == NFT - 1),
                        )
                    nc.scalar.copy(hT_sb[ot][:, bass.ts(rt, 128)], ht_ps[:])
            # s_i for local rows via PE: si[i] = sum_o hT[o,i] * a1[o]
            for it in range(NIT):
                si_ps = psx.tile([128, 1], F32, tag="mps")
                for ot in range(NOT_):
                    nc.tensor.matmul(
                        si_ps[:], hT_sb[ot][:, bass.ts(it, 128)], a1_col[ot][:],
                        start=(ot == 0), stop=(ot == NOT_ - 1),
                    )
                nc.scalar.copy(si_col[it][:], si_ps[:])
            # s_j for all nodes, replicated across partitions
            sj_ps = psx.tile([1, N], F32, tag="mps")
            for ot in range(NOT_):
                nc.tensor.matmul(
                    sj_ps[:], a2_col[ot][:], hT_sb[ot][:],
                    start=(ot == 0), stop=(ot == NOT_ - 1),
                )
            sj_row = sb1.tile([1, N], F32)
            nc.scalar.copy(sj_row[:], sj_ps[:])
            sjrep_ps = psx.tile([128, N], F32, tag="mps")
            nc.tensor.matmul(sjrep_ps[:], ones_row[:], sj_row[:])
            nc.scalar.copy(sj_rep[:], sjrep_ps[:])
            # masks (1.0 where adj > 0) and B = sj + si per row-block;
            # on Pool: these mid-stream DVE inserts were eating the
            # 0.45us/tile DVE-vs-DMA slack and backlogging the tail
            for it in range(NIT):
                nc.gpsimd.tensor_scalar(
                    mask_sb[it][:], adj_sb[it][:], 0.0, None, op0=ALU.is_gt
                )
                nc.gpsimd.tensor_scalar(
                    b_sb[it][:], sj_rep[:], si_col[it][:], None, op0=ALU.add
                )

        n_ed_issued = 0
        ot_tiles = []
        for it in range(NIT):
            # ---- edge stream: s_e via fp16 mult + pairwise tree ----
            for (j0, jw) in BLOCKS[it]:
                if n_ed_issued == 2:
                    input_dmas()
                n_ed_issued += 1
                if it == 0 and j0 == 0:
                    ed = pre_ed
                else:
                    ed = ed_pool.tile(
                        [128, jw * E_F], F16, tag=f"ed{jw}", name=f"ed{jw}",
                        bufs=(ED_BUFS if jw == 128 else 2),
                    )
                # the very last tile is the serial tail: land it as two
                # half DMAs into the same buffer so the first half's DVE
                # chain starts ~5us before the full tile has landed
                # (subtile deps track the halves independently)
                last_tile = (it == NIT - 1) and (j0 + jw == N)
                halves = 2 if last_tile else 1
                hw_ = jw // halves
                for hb in range(halves if (it == 0 and j0 == 0) else 0, halves):
                    nc.gpsimd.dma_start(
                        ed[:, hb * hw_ * E_F : (hb + 1) * hw_ * E_F],
                        edge_d[
                            bass.ts(it, 128), j0 + hb * hw_ : j0 + (hb + 1) * hw_, :
                        ].rearrange("p a b -> p (a b)"),
                    )
                prod = prod_pool.tile(
                    [128, jw * E_F], F16, tag=f"pr{jw}", name=f"pr{jw}"
                )
                t1 = prod_pool.tile([128, jw * 32], F16, tag=f"t1_{jw}", name=f"t1_{jw}")
                t2 = prod_pool.tile([128, jw * 16], F16, tag=f"t2_{jw}", name=f"t2_{jw}")
                t3 = prod_pool.tile([128, jw * 8], F16, tag=f"t3_{jw}", name=f"t3_{jw}")
                for hb in range(halves):
                    sl = slice(hb * hw_, (hb + 1) * hw_)
                    v0 = prod[:].rearrange("p (a b) -> p a b", b=E_F)[:, sl]
                    e0 = ed[:].rearrange("p (a b) -> p a b", b=E_F)[:, sl]
                    nc.vector.tensor_tensor(
                        v0, e0,
                        wcol[:, None, :].broadcast_to([128, hw_, E_F]),
                        ALU.mult,
                    )
                    v1 = t1[:].rearrange("p (a b) -> p a b", b=32)[:, sl]
                    nc.vector.tensor_tensor(v1, v0[:, :, 0:32], v0[:, :, 32:64], ALU.add)
                    v2 = t2[:].rearrange("p (a b) -> p a b", b=16)[:, sl]
                    nc.vector.tensor_tensor(v2, v1[:, :, 0:16], v1[:, :, 16:32], ALU.add)
                    v3 = t3[:].rearrange("p (a b) -> p a b", b=8)[:, sl]
                    nc.vector.tensor_tensor(v3, v2[:, :, 0:8], v2[:, :, 8:16], ALU.add)
                    with nc.allow_low_precision(reason="fp16 sum of 8 small partials"):
                        nc.vector.reduce_sum(
                            se_dve[it][:, j0 + hb * hw_ : j0 + (hb + 1) * hw_],
                            v3,
                            axis=mybir.AxisListType.X,
                        )

            if it == 0:
                setup2()

            # ---- softmax (no max-subtraction; exp biased into fp16 range) ----
            z = soft_pool.tile([128, N], F16, tag="z", bufs=1)
            zl = soft_pool.tile([128, N], F16, tag="zl", bufs=1)
            p = soft_pool.tile([128, N], F16, tag="p")
            pm = soft_pool.tile([128, N], F16, tag="pm")
            # last row-block: softmax in two segments so [0:384] runs while
            # the final edge tile is still landing; only the 128-wide
            # segment stays on the serial tail
            segs = [(0, 384), (384, 128)] if it == NIT - 1 else [(0, N)]
            for (s0, sw_) in segs:
                sl = slice(s0, s0 + sw_)
                nc.vector.tensor_tensor(z[:, sl], se_dve[it][:, sl], b_sb[it][:, sl], ALU.add)
                nc.vector.scalar_tensor_tensor(
                    out=zl[:, sl], in0=z[:, sl], scalar=ALPHA, in1=z[:, sl],
                    op0=ALU.mult, op1=ALU.max,
                )
                nc.scalar.activation(p[:, sl], zl[:, sl], AF.Exp, bias=expbias[:])
                nc.vector.tensor_tensor(pm[:, sl], p[:, sl], mask_sb[it][:, sl], ALU.mult)
            denom = soft_pool.tile([128, 1], F32, tag="den")
            nc.vector.reduce_sum(denom[:], pm[:], axis=mybir.AxisListType.X)
            rden = soft_pool.tile([128, 1], F32, tag="rden")
            nc.vector.reciprocal(rden[:], denom[:])

            # ---- h' = att @ h (fp16 PE path) ----
            hp_ps = hp_ps_pool.tile([128, OUT_F], F32)
            for jt in range(NJT):
                aps = att_ps_pool.tile([128, 128], F16)
                nc.tensor.transpose(aps[:], pm[:, bass.ts(jt, 128)], ident16[:])
                asb = attT_pool.tile([128, 128], F16)
                nc.scalar.copy(asb[:], aps[:])
                nc.tensor.matmul(
                    hp_ps[:], asb[:], h16_sb[jt][:],
                    start=(jt == 0), stop=(jt == NJT - 1),
                )

            # ---- normalize + ELU + store ----
            xx = out_pool.tile([128, OUT_F], F32, tag="xx", bufs=1)
            nc.scalar.mul(xx[:], hp_ps[:], rden[:])
            tmin = out_pool.tile([128, OUT_F], F32, tag="tm", bufs=1)
            nc.vector.tensor_scalar(tmin[:], xx[:], 0.0, None, op0=ALU.min)
            ex = out_pool.tile([128, OUT_F], F32, tag="ex", bufs=1)
            nc.scalar.activation(ex[:], tmin[:], AF.Exp)
            ot_sb = out_pool.tile([128, OUT_F], F32, tag="ot")
            nc.vector.scalar_tensor_tensor(
                out=ot_sb[:], in0=ex[:], scalar=-1.0, in1=xx[:], op0=ALU.add, op1=ALU.max
            )
            ot_tiles.append((it, ot_sb))

        # out stores ride the SWDGE queue too (a HWDGE store mid-span would
        # re-trigger the engine-79 queue-manager slowdown)
        for it, ot_sb in ot_tiles:
            nc.gpsimd.dma_start(out_d[bass.ts(it, 128), :], ot_sb[:])

    nc.compile()
    return nc


def _shard(x, edge, adj, W, W1, a):
    in_maps = []
    for c in range(N_CORES):
        bi, half = c // 2, c % 2
        r0 = half * R
        if r0:
            xb = np.roll(x[bi], -r0, axis=0)
            ed = np.roll(edge[bi, r0 : r0 + R], -r0, axis=1)
            ad = np.roll(adj[bi, r0 : r0 + R], -r0, axis=1)
        else:
            xb = x[bi]
            ed = edge[bi, 0:R]
            ad = adj[bi, 0:R]
        in_maps.append(
            {
                "edge_s": np.ascontiguousarray(ed),
                "adj_s": np.ascontiguousarray(ad),
                "x_b": np.ascontiguousarray(xb),
                "W": W,
                "W1": W1,
                "a": a,
            }
        )
    return in_maps


def kernel(x, edge, adj, W, W1, a, _trace=False):
    if "nc" not in _CACHE:
        _CACHE["nc"] = build_program()
    nc = _CACHE["nc"]

    x = np.asarray(x, dtype=np.float32)
    edge = np.asarray(edge, dtype=np.float32)
    adj = np.asarray(adj, dtype=np.float32)
    W = np.ascontiguousarray(np.asarray(W, dtype=np.float32))
    W1 = np.ascontiguousarray(np.asarray(W1, dtype=np.float32))
    a = np.ascontiguousarray(np.asarray(a, dtype=np.float32).reshape(3 * OUT_F, 1))

    in_maps = _shard(x, edge, adj, W, W1, a)
    res = run_bass_kernel_spmd(nc, in_maps, core_ids=list(range(N_CORES)), trace=_trace)
    out = np.empty((B, N, OUT_F), dtype=np.float32)
    for c in range(N_CORES):
        bi, half = c // 2, c % 2
        out[bi, half * R : (half + 1) * R] = res.results[c]["out_s"]
    if _trace:
        _CACHE["last_exec_time_ns"] = res.exec_time_ns
        _CACHE["last_res"] = res
    return out
